# revision 45
# baseline (speedup 1.0000x reference)
"""FFT-based DCT-II on 8 trn2 NeuronCores (rev L).

Per core (256 rows): Makhoul DCT->real-FFT, four-step radix-64x64, twiddles
folded into stage-2 tables, conjugate symmetry (66 stage-1 slots incl. two
zero columns), mid-transpose via DRAM roundtrip. fp16 operands, fp32 psum,
fp16 output (host converts back to fp32).

vs rev E: f1 merged into the x1 chunk-0 load (no 132B-descriptor DMA at
the head of the queue); T written in p-pairs and read full-width, all on
the sync queue/engine (the scalar engine's copyback time is the pipeline
pacer - it must not also pay ~0.7us per DMA issue); y staged in an SBUF
fp16 tile and written as four grouped DMAs on the scalar queue, which is
otherwise idle.
"""

import numpy as np

N = 4096
R = 2048
RPC = 256

_state = {}


def _tables():
    n1 = np.arange(64)[:, None].astype(np.float64)
    j = np.arange(33)[None, :].astype(np.float64)
    F1c = np.cos(2 * np.pi * n1 * j / 64)
    F1s = -np.sin(2 * np.pi * n1 * j / 64)
    F1 = np.concatenate([F1c, F1s], axis=1)  # [64, 66]; cols 33 & 65 are 0
    f1_np = np.vstack([F1, F1]).astype(np.float16)  # [128, 66]

    n2v = np.arange(64)[:, None].astype(np.float64)
    k2v = np.arange(64)[None, :].astype(np.float64)

    def HH_single(k1):
        k = 64 * k2v + k1
        Gc = np.cos(2 * np.pi * n2v * k / N)
        Gs = -np.sin(2 * np.pi * n2v * k / N)
        cosE = np.cos(np.pi * k / (2 * N))
        sinE = np.sin(np.pi * k / (2 * N))
        sigma = 1.0 if k1 <= 32 else -1.0
        H1 = cosE * Gc + sinE * Gs
        H2 = sigma * (sinE * Gc - cosE * Gs)
        return np.concatenate([H1, H2], axis=0)  # [128, 64]

    HH = np.zeros((33, 128, 128))
    for a in range(1, 32):
        HH[a][:, :64] = HH_single(a)
        HH[a][:, 64:] = HH_single(64 - a)
    HH[0][:, :64] = HH_single(0)
    HH[32][:, 64:] = HH_single(32)
    # t2 partitions come from the (n c) DMA merge: p = 2*n2 + c
    rowperm = np.empty(128, dtype=np.int64)
    for n2 in range(64):
        for c in range(2):
            rowperm[2 * n2 + c] = c * 64 + n2
    HH = HH[:, rowperm, :]
    hh_np = HH.transpose(1, 0, 2).astype(np.float16).copy()  # [128, 33, 128]

    k1_arr = np.empty(64, dtype=np.int64)
    for a in range(32):
        k1_arr[2 * a] = a
        k1_arr[2 * a + 1] = (64 - a) if a > 0 else 32
    return f1_np, hh_np, k1_arr


def _t2_slice(t2_tiles, a):
    if a == 32:
        return t2_tiles[3][:, 8, :]
    return t2_tiles[a // 8][:, a % 8, :]


def _build():
    import concourse.tile as tile
    from concourse import bacc, mybir

    f16 = mybir.dt.float16
    f32 = mybir.dt.float32

    nc = bacc.Bacc("TRN2", target_bir_lowering=False, debug=False, num_devices=8)
    # x1 carries f1 in its first 66 columns.
    x1_d = nc.dram_tensor("x1", [128, 66 + 8192], f16, kind="ExternalInput").ap()
    hh_d = nc.dram_tensor("hh", [128, 33, 128], f16, kind="ExternalInput").ap()
    y_d = nc.dram_tensor("y", [128, 32, 256], f16, kind="ExternalOutput").ap()

    with tile.TileContext(nc) as tc:
        with (
            tc.tile_pool(name="const", bufs=1) as const,
            tc.tile_pool(name="data", bufs=1) as data,
            tc.tile_pool(name="dram", bufs=1, space="DRAM") as dram,
            tc.tile_pool(name="ps1", bufs=3, space="PSUM") as ps1,
            tc.tile_pool(name="ps2", bufs=2, space="PSUM") as ps2,
        ):
            hh_sb = const.tile([128, 33, 128], f16)

            # x1 on the sync queue, dense back-to-back; the first chunk is
            # small (f1 + p0's columns) so stage 1 starts ~2us earlier; hh
            # follows on the same queue (a second concurrent stream would
            # halve x1's bandwidth).
            xw = [1090, 1024, 2048, 2048, 2048]
            xoff = [0, 1090, 2114, 4162, 6210]
            x1_g = []
            for g in range(5):
                xg = data.tile([128, xw[g]], f16, name=f"x1_{g}")
                nc.sync.dma_start(
                    xg[:], x1_d[:, xoff[g] : xoff[g] + xw[g]]
                )
                x1_g.append(xg)
            f1_sb = x1_g[0]  # cols 0:66 are f1

            # hh rides the scalar queue (idle until the T2 reads), delayed
            # past the x1 stream so it never contends with it; the sync
            # queue then starts draining T writes immediately after x1.
            with tc.tile_wait_until(0.012):
                nc.scalar.dma_start(hh_sb[:], hh_d)

            def x_slice(f, h):
                # column range of f-group f: [66+512f, 66+512f+512)
                lo = 66 + 512 * f
                for g in range(5):
                    if xoff[g] <= lo and lo + 512 <= xoff[g] + xw[g]:
                        s = lo - xoff[g]
                        return x1_g[g][64 * h : 64 * h + 64, s : s + 512]
                raise AssertionError(f)

            t_dram = dram.tile([64, 2, 33, 256], f16)  # [n2, c, m, r]
            t_sb_g = [
                data.tile([66, 16, 256], f16, name=f"tsb_{g}") for g in range(4)
            ]

            # stage 1: f in [0,16), psum tile per (p=f//2, h) holds 2 MMs.
            # h-alternating emission so adjacent MMs hit different PE row
            # groups. Copybacks alternate vector/scalar; T written in
            # p-pairs on the sync queue (scalar must stay copy-only).
            cb = 0
            for p in range(8):
                tiles = [
                    ps1.tile([66, 2, 512], f32, name=f"s1ps_{p}_{h}", tag="s1ps")
                    for h in range(2)
                ]
                for j in range(2):
                    for h in range(2):
                        nc.tensor.matmul(
                            tiles[h][:, j, :],
                            f1_sb[64 * h : 64 * h + 64, 0:66],
                            x_slice(2 * p + j, h),
                            start=True,
                            stop=True,
                        )
                for h in range(2):
                    dst = t_sb_g[p // 2][
                        :, (p % 2) * 8 : (p % 2) * 8 + 8, 128 * h : 128 * h + 128
                    ]
                    src = tiles[h][:].rearrange("s j (a b) -> s (j a) b", a=4)
                    if cb % 2 == 0:
                        nc.vector.tensor_copy(dst, src)
                    else:
                        nc.scalar.copy(dst, src)
                    cb += 1
                # T writes: pairs for p0-5, singles for p6/p7 so the last
                # (read-gating) write is small and lands right after p7's
                # copyback
                if p in (1, 3, 5):
                    p0 = p - 1
                    nc.sync.dma_start(
                        t_dram[8 * p0 : 8 * p0 + 16].rearrange(
                            "n c m r -> (c m) n r"
                        ),
                        t_sb_g[p0 // 2][:, :, :],
                    )
                elif p in (6, 7):
                    nc.sync.dma_start(
                        t_dram[8 * p : 8 * p + 8].rearrange(
                            "n c m r -> (c m) n r"
                        ),
                        t_sb_g[3][:, (p % 2) * 8 : (p % 2) * 8 + 8, :],
                    )

            # T2 read in m-chunks, full 128-partition width, on the sync
            # queue right behind the T writes. m=32 rides contiguously with
            # chunk 3 (m 24..32); slot a=0 is processed last in stage 2.
            t2_tiles = [
                data.tile([128, 9 if j == 3 else 8, 256], f16, name=f"t2_{j}")
                for j in range(4)
            ]
            t_rd = t_dram[:].rearrange("n c m r -> (n c) m r")
            rd_eng = [nc.sync, nc.scalar, nc.sync, nc.scalar]
            for j in range(4):
                w = 9 if j == 3 else 8
                rd_eng[j].dma_start(
                    t2_tiles[j][:, 0:w, :], t_rd[:, 8 * j : 8 * j + w, :]
                )

            # stage 2: 16 psum tiles, each two a's; a=0 accumulates m=0 and
            # m=32. Copybacks (with fp32->fp16 cast) go into a staging tile;
            # y leaves in four grouped DMAs on the otherwise-idle scalar
            # queue.
            y_sb = data.tile([128, 32, 256], f16)
            for q in list(range(1, 16)) + [0]:
                ps = ps2.tile([128, 512], f32)
                for i in range(2):
                    a = 2 * q + i
                    out = ps[:, 256 * i : 256 * i + 256]
                    if a == 0:
                        nc.tensor.matmul(
                            out, hh_sb[:, 0, :], _t2_slice(t2_tiles, 0),
                            start=True, stop=False,
                        )
                        nc.tensor.matmul(
                            out, hh_sb[:, 32, :], _t2_slice(t2_tiles, 32),
                            start=False, stop=True,
                        )
                    else:
                        nc.tensor.matmul(
                            out, hh_sb[:, a, :], _t2_slice(t2_tiles, a),
                            start=True, stop=True,
                        )
                dst = y_sb[:, 2 * q : 2 * q + 2, :]
                src = ps[:].rearrange("p (a r) -> p a r", a=2)
                if q % 2 == 0:
                    nc.vector.tensor_copy(dst, src)
                else:
                    nc.scalar.copy(dst, src)
                if q in (4, 8, 12, 15):
                    lo, w = {4: (2, 8), 8: (10, 8), 12: (18, 8), 15: (26, 6)}[q]
                    eng = nc.scalar if q in (4, 8) else nc.sync
                    eng.dma_start(
                        y_d[:, lo : lo + w, :], y_sb[:, lo : lo + w, :]
                    )
                elif q == 0:
                    nc.scalar.dma_start(y_d[:, 0:2, :], y_sb[:, 0:2, :])

    nc.compile()
    return nc


def _pack_x1(x_rows, f1_np):
    v = np.empty_like(x_rows)
    v[:, : N // 2] = x_rows[:, 0::2]
    v[:, N // 2 :] = x_rows[:, 1::2][:, ::-1]
    x1 = v.reshape(2, 128, 64, 64).transpose(0, 2, 3, 1).reshape(128, 8192)
    return np.ascontiguousarray(
        np.concatenate([f1_np, x1.astype(np.float16)], axis=1)
    )


def kernel(x, _trace: bool = False):
    from concourse.bass_utils import run_bass_kernel_spmd

    x = np.asarray(x, dtype=np.float32)
    assert x.shape == (R, N)
    if "nc" not in _state:
        _state["nc"] = _build()
        _state["tables"] = _tables()
    nc = _state["nc"]
    f1_np, hh_np, k1_arr = _state["tables"]

    in_maps = []
    for c in range(8):
        in_maps.append(
            {
                "x1": _pack_x1(x[c * RPC : (c + 1) * RPC], f1_np),
                "hh": hh_np,
            }
        )

    res = run_bass_kernel_spmd(nc, in_maps, list(range(8)), trace=_trace)

    y = np.empty((R, N), dtype=np.float32)
    for c in range(8):
        ydev = res.results[c]["y"].astype(np.float32)  # [128, 32, 256] fp16
        # partitions = (d, k2); slot index (a, d) -> k1 = k1_arr[2a+d]
        perm = ydev.reshape(2, 64, 32, 256).transpose(3, 1, 2, 0).reshape(RPC, 64, 64)
        yc = np.empty((RPC, 64, 64), dtype=np.float32)
        yc[:, :, k1_arr] = perm
        y[c * RPC : (c + 1) * RPC] = yc.reshape(RPC, N)
    if _trace:
        _state["last_result"] = res
    return y


# revision 47
# speedup vs baseline: 1.2024x; 1.2024x over previous
"""FFT-based DCT-II on 8 trn2 NeuronCores (rev L).

Per core (256 rows): Makhoul DCT->real-FFT, four-step radix-64x64, twiddles
folded into stage-2 tables, conjugate symmetry (66 stage-1 slots incl. two
zero columns), mid-transpose via DRAM roundtrip. fp16 operands, fp32 psum,
fp16 output (host converts back to fp32).

vs rev E: f1 merged into the x1 chunk-0 load (no 132B-descriptor DMA at
the head of the queue); T written in p-pairs and read full-width, all on
the sync queue/engine (the scalar engine's copyback time is the pipeline
pacer - it must not also pay ~0.7us per DMA issue); y staged in an SBUF
fp16 tile and written as four grouped DMAs on the scalar queue, which is
otherwise idle.
"""

import numpy as np

N = 4096
R = 2048
RPC = 256

_state = {}


def _tables():
    n1 = np.arange(64)[:, None].astype(np.float64)
    j = np.arange(33)[None, :].astype(np.float64)
    F1c = np.cos(2 * np.pi * n1 * j / 64)
    F1s = -np.sin(2 * np.pi * n1 * j / 64)
    F1 = np.concatenate([F1c, F1s], axis=1)  # [64, 66]; cols 33 & 65 are 0
    f1_np = np.vstack([F1, F1]).astype(np.float16)  # [128, 66]

    n2v = np.arange(64)[:, None].astype(np.float64)
    k2v = np.arange(64)[None, :].astype(np.float64)

    def HH_single(k1):
        k = 64 * k2v + k1
        Gc = np.cos(2 * np.pi * n2v * k / N)
        Gs = -np.sin(2 * np.pi * n2v * k / N)
        cosE = np.cos(np.pi * k / (2 * N))
        sinE = np.sin(np.pi * k / (2 * N))
        sigma = 1.0 if k1 <= 32 else -1.0
        H1 = cosE * Gc + sinE * Gs
        H2 = sigma * (sinE * Gc - cosE * Gs)
        return np.concatenate([H1, H2], axis=0)  # [128, 64]

    HH = np.zeros((33, 128, 128))
    for a in range(1, 32):
        HH[a][:, :64] = HH_single(a)
        HH[a][:, 64:] = HH_single(64 - a)
    HH[0][:, :64] = HH_single(0)
    HH[32][:, 64:] = HH_single(32)
    # t2 partitions come from the (n c) DMA merge: p = 2*n2 + c
    rowperm = np.empty(128, dtype=np.int64)
    for n2 in range(64):
        for c in range(2):
            rowperm[2 * n2 + c] = c * 64 + n2
    HH = HH[:, rowperm, :]
    hh_np = HH.transpose(1, 0, 2).astype(np.float16).copy()  # [128, 33, 128]

    k1_arr = np.empty(64, dtype=np.int64)
    for a in range(32):
        k1_arr[2 * a] = a
        k1_arr[2 * a + 1] = (64 - a) if a > 0 else 32
    return f1_np, hh_np, k1_arr


def _t2_slice(t2_tiles, a):
    if a == 32:
        return t2_tiles[3][:, 8, :]
    return t2_tiles[a // 8][:, a % 8, :]


def _build():
    import concourse.tile as tile
    from concourse import bacc, mybir

    f16 = mybir.dt.float16
    f32 = mybir.dt.float32

    nc = bacc.Bacc("TRN2", target_bir_lowering=False, debug=False, num_devices=8)
    # x1 carries f1 in its first 66 columns.
    x1_d = nc.dram_tensor("x1", [128, 66 + 8192], f16, kind="ExternalInput").ap()
    hh_d = nc.dram_tensor("hh", [128, 33, 128], f16, kind="ExternalInput").ap()
    y_d = nc.dram_tensor("y", [128, 32, 256], f16, kind="ExternalOutput").ap()

    with tile.TileContext(nc) as tc:
        with (
            tc.tile_pool(name="const", bufs=1) as const,
            tc.tile_pool(name="data", bufs=1) as data,
            tc.tile_pool(name="dram", bufs=1, space="DRAM") as dram,
            tc.tile_pool(name="ps1", bufs=3, space="PSUM") as ps1,
            tc.tile_pool(name="ps2", bufs=2, space="PSUM") as ps2,
        ):
            hh_sb = const.tile([128, 33, 128], f16)

            # x1 on the sync queue, dense back-to-back; the first chunk is
            # small (f1 + p0's columns) so stage 1 starts ~2us earlier; hh
            # follows on the same queue (a second concurrent stream would
            # halve x1's bandwidth).
            xw = [1090, 1024, 2048, 2048, 2048]
            xoff = [0, 1090, 2114, 4162, 6210]
            x1_g = []
            for g in range(5):
                xg = data.tile([128, xw[g]], f16, name=f"x1_{g}")
                nc.sync.dma_start(
                    xg[:], x1_d[:, xoff[g] : xoff[g] + xw[g]]
                )
                x1_g.append(xg)
            nc.sync.dma_start(hh_sb[:], hh_d)
            f1_sb = x1_g[0]  # cols 0:66 are f1

            def x_slice(f, h):
                # column range of f-group f: [66+512f, 66+512f+512)
                lo = 66 + 512 * f
                for g in range(5):
                    if xoff[g] <= lo and lo + 512 <= xoff[g] + xw[g]:
                        s = lo - xoff[g]
                        return x1_g[g][64 * h : 64 * h + 64, s : s + 512]
                raise AssertionError(f)

            t_dram = dram.tile([64, 2, 33, 256], f16)  # [n2, c, m, r]
            t_sb_g = [
                data.tile([66, 16, 256], f16, name=f"tsb_{g}") for g in range(4)
            ]

            # stage 1: f in [0,16), psum tile per (p=f//2, h) holds 2 MMs.
            # h-alternating emission so adjacent MMs hit different PE row
            # groups. Copybacks alternate vector/scalar; T written in
            # p-pairs on the sync queue (scalar must stay copy-only).
            cb = 0
            for p in range(8):
                tiles = [
                    ps1.tile([66, 2, 512], f32, name=f"s1ps_{p}_{h}", tag="s1ps")
                    for h in range(2)
                ]
                for j in range(2):
                    for h in range(2):
                        nc.tensor.matmul(
                            tiles[h][:, j, :],
                            f1_sb[64 * h : 64 * h + 64, 0:66],
                            x_slice(2 * p + j, h),
                            start=True,
                            stop=True,
                        )
                for h in range(2):
                    dst = t_sb_g[p // 2][
                        :, (p % 2) * 8 : (p % 2) * 8 + 8, 128 * h : 128 * h + 128
                    ]
                    src = tiles[h][:].rearrange("s j (a b) -> s (j a) b", a=4)
                    if cb % 2 == 0:
                        nc.vector.tensor_copy(dst, src)
                    else:
                        nc.scalar.copy(dst, src)
                    cb += 1
                # T writes: pairs for p0-5, singles for p6/p7 so the last
                # (read-gating) write is small and lands right after p7's
                # copyback
                if p in (1, 3, 5):
                    p0 = p - 1
                    nc.sync.dma_start(
                        t_dram[8 * p0 : 8 * p0 + 16].rearrange(
                            "n c m r -> (c m) n r"
                        ),
                        t_sb_g[p0 // 2][:, :, :],
                    )
                elif p in (6, 7):
                    nc.sync.dma_start(
                        t_dram[8 * p : 8 * p + 8].rearrange(
                            "n c m r -> (c m) n r"
                        ),
                        t_sb_g[3][:, (p % 2) * 8 : (p % 2) * 8 + 8, :],
                    )

            # T2 read in m-chunks, full 128-partition width, on the sync
            # queue right behind the T writes. m=32 rides contiguously with
            # chunk 3 (m 24..32); slot a=0 is processed last in stage 2.
            t2_tiles = [
                data.tile([128, 9 if j == 3 else 8, 256], f16, name=f"t2_{j}")
                for j in range(4)
            ]
            t_rd = t_dram[:].rearrange("n c m r -> (n c) m r")
            rd_eng = [nc.sync, nc.scalar, nc.sync, nc.scalar]
            for j in range(4):
                w = 9 if j == 3 else 8
                rd_eng[j].dma_start(
                    t2_tiles[j][:, 0:w, :], t_rd[:, 8 * j : 8 * j + w, :]
                )

            # stage 2: 16 psum tiles, each two a's; a=0 accumulates m=0 and
            # m=32. Copybacks (with fp32->fp16 cast) go into a staging tile;
            # y leaves in four grouped DMAs on the otherwise-idle scalar
            # queue.
            y_sb = data.tile([128, 32, 256], f16)
            for q in list(range(1, 16)) + [0]:
                ps = ps2.tile([128, 512], f32)
                for i in range(2):
                    a = 2 * q + i
                    out = ps[:, 256 * i : 256 * i + 256]
                    if a == 0:
                        nc.tensor.matmul(
                            out, hh_sb[:, 0, :], _t2_slice(t2_tiles, 0),
                            start=True, stop=False,
                        )
                        nc.tensor.matmul(
                            out, hh_sb[:, 32, :], _t2_slice(t2_tiles, 32),
                            start=False, stop=True,
                        )
                    else:
                        nc.tensor.matmul(
                            out, hh_sb[:, a, :], _t2_slice(t2_tiles, a),
                            start=True, stop=True,
                        )
                dst = y_sb[:, 2 * q : 2 * q + 2, :]
                src = ps[:].rearrange("p (a r) -> p a r", a=2)
                if q % 2 == 0:
                    nc.vector.tensor_copy(dst, src)
                else:
                    nc.scalar.copy(dst, src)
                if q in (4, 8, 12, 13, 14, 15):
                    # tail groups shrink to 2 slots so the final (chain-
                    # gated) y write is small and drains immediately
                    lo, w = {
                        4: (2, 8), 8: (10, 8), 12: (18, 8),
                        13: (26, 2), 14: (28, 2), 15: (30, 2),
                    }[q]
                    eng = nc.scalar if q in (4, 8) else nc.sync
                    eng.dma_start(
                        y_d[:, lo : lo + w, :], y_sb[:, lo : lo + w, :]
                    )
                elif q == 0:
                    nc.scalar.dma_start(y_d[:, 0:2, :], y_sb[:, 0:2, :])

    nc.compile()
    return nc


def _pack_x1(x_rows, f1_np):
    v = np.empty_like(x_rows)
    v[:, : N // 2] = x_rows[:, 0::2]
    v[:, N // 2 :] = x_rows[:, 1::2][:, ::-1]
    x1 = v.reshape(2, 128, 64, 64).transpose(0, 2, 3, 1).reshape(128, 8192)
    return np.ascontiguousarray(
        np.concatenate([f1_np, x1.astype(np.float16)], axis=1)
    )


def kernel(x, _trace: bool = False):
    from concourse.bass_utils import run_bass_kernel_spmd

    x = np.asarray(x, dtype=np.float32)
    assert x.shape == (R, N)
    if "nc" not in _state:
        _state["nc"] = _build()
        _state["tables"] = _tables()
    nc = _state["nc"]
    f1_np, hh_np, k1_arr = _state["tables"]

    in_maps = []
    for c in range(8):
        in_maps.append(
            {
                "x1": _pack_x1(x[c * RPC : (c + 1) * RPC], f1_np),
                "hh": hh_np,
            }
        )

    res = run_bass_kernel_spmd(nc, in_maps, list(range(8)), trace=_trace)

    y = np.empty((R, N), dtype=np.float32)
    for c in range(8):
        ydev = res.results[c]["y"].astype(np.float32)  # [128, 32, 256] fp16
        # partitions = (d, k2); slot index (a, d) -> k1 = k1_arr[2a+d]
        perm = ydev.reshape(2, 64, 32, 256).transpose(3, 1, 2, 0).reshape(RPC, 64, 64)
        yc = np.empty((RPC, 64, 64), dtype=np.float32)
        yc[:, :, k1_arr] = perm
        y[c * RPC : (c + 1) * RPC] = yc.reshape(RPC, N)
    if _trace:
        _state["last_result"] = res
    return y
